# revision 23
# baseline (speedup 1.0000x reference)
"""Encoder-decoder attention kernel for Trainium2, 8 NeuronCores.

Sharding: batch (B=8) data-parallel, one batch element per core; weights
replicated. Per core (S=Sq=Sk=1024, H=1024, NH=16, D=64):

  phase A: transpose X_dec, X_enc via PE -> x_dec_t/x_enc_t [h,s] f32r
           (+ fp16 copy of X_enc^T for the V projection); input tiles
           double-buffered, DMAs alternate sync/scalar queues; PSUM->SBUF
           copies batched 4-wide (strided) to amortize per-op overhead
  per pair p (heads 2p, 2p+1), prep woven into the qt loop as half-sized
  chunks so the PE never idles:
    Q^T/K^T proj: fp32r matmuls, w tiles raw-bit f32r DMA, accumulated
      in [128,512] PSUM halves; Q written fused [128(2-head nd),512]
      so scores contract K=64 against partition-sliced k_s
    V proj: fp16 (x_enc_h stationary, w tiles fp16 straight from DRAM)
    scores: per qt, 4 fp32r K=64 matmuls (2 heads x 2 k-halves, h0/h1
      in disjoint row groups) into two [128,1024] PSUM tiles; DVE
      negated-max reduce per head
    softmax: ACT exp(bias=-max, accum_out=rowsum into a per-blk stat
      column) -> UNNORMALIZED fp16 P -> per-qt hw DMA transpose on Sync
    normalization is deferred: per blk the 4 rowsums [128,4] get one
      batched DVE reciprocal + fp16 cast, one tiny DMA transpose
      ([128,16]->[16,128], q moves to the free axis), a K=1 ones-matmul
      broadcasts recip across d-partitions into the spare half of the
      PV PSUM tile, and the PV->concat copy becomes a DVE multiply
    O^T = V^T P^T in fp16, ev/od heads col-tiled, emitted one block lag
  phase D: outT = W_out concat + b with W_out slices stationary (fp16
    from DRAM), concat moving; bias folded into PSUM via a K=1
    ones-matmul; host transposes the returned out^T
  HAM: the PE clock-gate re-throttles after idle windows; zero-data bf16
    pump matmuls are chained onto the PSUM tile rotation and onto p_e
    (post-exp) so PE activity tracks the softmax chain.

Precision: fp32r (~1.5e-4 mult rounding) through Q/K/scores gives score
abs err ~0.06; final rel err ~2e-3 vs the 2e-2 gate. P and V run in
fp16. 1/sqrt(D)=1/8 folded into W_query on the host.
"""
import sys

sys.path.insert(0, "/opt/trn_rl_repo")

import numpy as np

B = 8
S = 1024   # Sq == Sk
H = 1024
NH = 16
D = 64
P = 128
HT = H // P    # 8 h-tiles
ST = S // P    # 8 s-tiles == k-tiles
NP = NH // 2   # 8 head pairs
QB = 256       # q-block width for the P@V moving dim
NB = S // QB   # 4 q-blocks
QTB = QB // P  # 2 q-tiles per block


def build():
    import concourse.mybir as mybir
    import concourse.tile as tile
    from concourse import bacc
    from concourse.masks import make_identity

    f32 = mybir.dt.float32
    f32r = mybir.dt.float32r
    f16 = mybir.dt.float16
    AX = mybir.AxisListType.X
    OP = mybir.AluOpType
    AF = mybir.ActivationFunctionType

    nc = bacc.Bacc(trn_type="TRN2", target_bir_lowering=False, debug=False)

    xd_d = nc.dram_tensor("xd", [S, H], f32, kind="ExternalInput").ap()
    xe_d = nc.dram_tensor("xe", [S, H], f32, kind="ExternalInput").ap()
    # f32r raw bits == f32 bits (verified on hw); lets plain DMA feed
    # fp32r matmuls directly
    wqt_d = nc.dram_tensor("wqt", [H, H], f32r, kind="ExternalInput").ap()  # [h, nd] (pre-scaled 1/8)
    wkt_d = nc.dram_tensor("wkt", [H, H], f32r, kind="ExternalInput").ap()  # [h, nd]
    wvt_d = nc.dram_tensor("wvt", [H, H], f16, kind="ExternalInput").ap()   # [h, nd] fp16
    wot_d = nc.dram_tensor("wot", [H, H], f16, kind="ExternalInput").ap()   # [nd, h_out] fp16
    bias_d = nc.dram_tensor("bias", [1, H], f16, kind="ExternalInput").ap()
    sel_d = nc.dram_tensor("sel", [2, P], f16, kind="ExternalInput").ap()
    # output is stored transposed ([h_out, q]); the host transposes back
    out_d = nc.dram_tensor("out", [H, S], f32, kind="ExternalOutput").ap()

    from contextlib import ExitStack
    with tile.TileContext(nc) as tc:
        with ExitStack() as ctx:
            big = ctx.enter_context(tc.tile_pool(name="big", bufs=1))
            qtp = ctx.enter_context(tc.tile_pool(name="qt", bufs=1))
            ksp = ctx.enter_context(tc.tile_pool(name="ks", bufs=2))
            vpp = ctx.enter_context(tc.tile_pool(name="vp", bufs=2))
            ccp = ctx.enter_context(tc.tile_pool(name="cc", bufs=NP))
            xehp = ctx.enter_context(tc.tile_pool(name="xeh", bufs=1))
            xinp = ctx.enter_context(tc.tile_pool(name="xin", bufs=2))
            pep = ctx.enter_context(tc.tile_pool(name="pe", bufs=3))
            ptp = ctx.enter_context(tc.tile_pool(name="pt", bufs=4))
            wtp = ctx.enter_context(tc.tile_pool(name="wt", bufs=2))
            wvcp = ctx.enter_context(tc.tile_pool(name="wvc", bufs=8))
            worp = ctx.enter_context(tc.tile_pool(name="wor", bufs=2))
            osbp = ctx.enter_context(tc.tile_pool(name="osb", bufs=2))
            constp = ctx.enter_context(tc.tile_pool(name="const", bufs=1))
            statp = ctx.enter_context(tc.tile_pool(name="stat", bufs=16))
            rsp = ctx.enter_context(tc.tile_pool(name="rs", bufs=4))
            rstp = ctx.enter_context(tc.tile_pool(name="rst", bufs=4))
            mlp = ctx.enter_context(tc.tile_pool(name="ml", bufs=2))
            psp = ctx.enter_context(tc.tile_pool(name="ps", bufs=2, space="PSUM"))
            psSp = ctx.enter_context(tc.tile_pool(name="psS", bufs=3, space="PSUM"))

            def stat():
                return statp.tile([P, 1], f32, tag="stat", name="stat")

            # ---- constants ----
            ident = constp.tile([P, P], f32)
            make_identity(nc, ident[:])
            ones16 = constp.tile([1, 512], f16)
            nc.vector.memset(ones16[:], 1.0)
            bias16 = constp.tile([1, H], f16)
            nc.scalar.dma_start(bias16[:], bias_d)
            zeros = constp.tile([P, 256], f32)
            nc.vector.memset(zeros[:], 0.0)
            zb = zeros[:].bitcast(mybir.dt.bfloat16)  # [P, 512] bf16
            # persistent fp16 rowsum-recip staging slots (memset once so
            # the pad columns of the tiny transposes are initialized);
            # 2 per blk (one per qtb), double-buffered
            rs16_slots = []
            for sl in range(4):
                t = constp.tile([P, P], f16, name=f"rs16_{sl}")
                nc.vector.memset(t[:], 1.0)
                rs16_slots.append(t)
            # head-selector for the recip broadcast: row0 -> d 0:64 (h0),
            # row1 -> d 64:128 (h1); loaded from DRAM (partition-1 memsets
            # are rejected by the BIR verifier)
            sel2 = constp.tile([2, P], f16, name="sel2")
            nc.scalar.dma_start(sel2[:], sel_d)

            # HAM pump: PE transposes don't count as activity for the HAM
            # clock gate, and chain stalls would otherwise cross the ~3.4us
            # MID window and re-throttle to 1.2 GHz. pump_into() fires a
            # tiny zero-data bf16 matmul into a PSUM region about to be
            # overwritten anyway; the WAW dep makes it run exactly when the
            # buffer rotates, spreading PE activity through the chain.
            def pump_into(ps_ap, n=256):
                nc.tensor.matmul(ps_ap[0:64, 0:n], zb[:, 0:64], zb[:, 0:n],
                                 start=True, stop=True)

            def pstile(pump=True):
                t = psp.tile([P, 512], f32, tag="ps", name="ps")
                if pump:
                    pump_into(t[:], 128)
                return t

            def pstileS(pump=True):
                t = psSp.tile([P, S], f32, tag="psS", name="psS")
                if pump:
                    pump_into(t[:], 256)
                return t

            # dense burst into a throwaway PSUM tile to flip HAM to K=8/8
            def pump_burst(n):
                t = psSp.tile([P, S], f32, tag="psS", name="hampump")
                for _ in range(n):
                    nc.tensor.matmul(t[:, 0:512], zb[:, 0:128], zb[:, 0:512],
                                     start=True, stop=True)

            # warmup transpose absorbs the gpsimd(identity) dep on PE
            warm = pstile(pump=False)
            nc.tensor.transpose(warm[:, 0:P], ident[:], ident[:])

            # persistent fused Q tiles (both heads stacked on partitions):
            # rows 0:64 head-even d, 64:128 head-odd d; 2 slots rotate
            q_slots = [qtp.tile([P, S], f32r, name=f"qp_{sl}")
                       for sl in range(2)]

            # ---- block weight loads (one DMA per pair-projection) ----
            q_wt = {}
            k_wt = {}

            def load_wt(dst_map, dram, p, tag):
                wt = wtp.tile([P, HT, P], f32r, tag=tag, name=tag)
                nc.gpsimd.dma_start(
                    wt[:],
                    dram[:, p * P:(p + 1) * P]
                    .rearrange("(j q) c -> q j c", j=HT))
                dst_map[p] = wt

            # ---- prep chunks (emitted JIT, ~half-chunk per qt slot) ----
            q_t = {}
            k_s_next = [None]
            v2_next = [None]

            def q_chunks(p):
                def half(nn):
                    ns = slice(nn * 512, (nn + 1) * 512)
                    psh = pstile()
                    w = q_wt[p]
                    for j in range(HT):
                        nc.tensor.matmul(
                            psh[:], w[:, j, :], x_dec_t[:, j, ns],
                            start=(j == 0), stop=(j == HT - 1))
                    nc.scalar.activation(q_slots[p % 2][:, ns], psh[:],
                                         AF.Copy)

                def c1():
                    q_t[p] = q_slots[p % 2]
                    half(0)

                def c2():
                    half(1)

                return [c1, c2]

            def k_chunks(p):
                box = {}

                def half(nn):
                    ns = slice(nn * 512, (nn + 1) * 512)
                    psh = pstile()
                    w = k_wt[p]
                    for j in range(HT):
                        nc.tensor.matmul(
                            psh[:], w[:, j, :], x_enc_t[:, j, ns],
                            start=(j == 0), stop=(j == HT - 1))
                    nc.scalar.activation(box['k'][:, ns], psh[:], AF.Copy)

                def c1():
                    box['k'] = ksp.tile([P, S], f32r, tag="ks", name="ksb")
                    half(0)

                def c2():
                    half(1)
                    k_s_next[0] = box['k']

                return [c1, c2]

            def v_chunks(p):
                # group of 4 pairs (nd cols p*128..(p+4)*128); 8 pieces,
                # one k-tile each; fp16 w tiles DMA'd straight from DRAM
                box = {'wh': None}

                def piece(kt_i):
                    def ci():
                        if box['wh'] is None:
                            box['wh'] = []
                            for j in range(HT):
                                wh = wvcp.tile([P, 512], f16, tag="wvh",
                                               name="wvh")
                                nc.gpsimd.dma_start(
                                    wh[:],
                                    wvt_d[j * P:(j + 1) * P,
                                          p * P:(p + 4) * P])
                                box['wh'].append(wh)
                        if kt_i == 0:
                            box['v2'] = vpp.tile([P, ST, 512], f16,
                                                 tag="vp", name="v2")
                            v2_next[0] = box['v2']
                        psh = pstile()
                        for j in range(HT):
                            nc.tensor.matmul(
                                psh[:],
                                x_enc_h[:, j, kt_i * P:(kt_i + 1) * P],
                                box['wh'][j][:],
                                start=(j == 0), stop=(j == HT - 1))
                        nc.vector.tensor_copy(box['v2'][:, kt_i, :], psh[:])
                    return ci

                return [piece(i) for i in range(ST)]

            # ---- phase A + prologue, interleaved ----
            x_dec_t = big.tile([P, HT, S], f32r, name="xdt")[:]
            x_enc_t = big.tile([P, HT, S], f32r, name="xet")[:]
            x_enc_h = xehp.tile([P, HT, S], f16, name="xeh")[:]
            load_wt(q_wt, wqt_d, 0, "qw")
            load_wt(k_wt, wkt_d, 0, "kw")
            load_wt(q_wt, wqt_d, 1, "qw")
            load_wt(k_wt, wkt_d, 1, "kw")
            qc0 = q_chunks(0)
            kc0 = k_chunks(0)
            vA = v_chunks(0)

            def phase_a_tile(xt, src, i, enc):
                xin = xinp.tile([P, H], f32, tag="xin")
                # alternate input DMAs across the two hwdge queues
                eng = nc.sync if i % 2 == 0 else nc.scalar
                eng.dma_start(xin[:], src[i * P:(i + 1) * P, :])
                for g in range(2):
                    pst = pstile(pump=False)
                    pump_into(pst[:], 128)
                    for t in range(4):
                        j = g * 4 + t
                        nc.tensor.transpose(
                            pst[:, t * P:(t + 1) * P],
                            xin[:, j * P:(j + 1) * P], ident[:])
                    # batched 4-wide strided copies (one op per group)
                    src4 = pst[:].rearrange("p (a b) -> p a b", a=4)
                    nc.vector.tensor_copy(
                        xt[:, g * 4:(g + 1) * 4, i * P:(i + 1) * P], src4)
                    if enc:
                        nc.scalar.activation(
                            x_enc_h[:, g * 4:(g + 1) * 4,
                                    i * P:(i + 1) * P], src4, AF.Copy)

            with nc.named_scope("phaseA"):
                # dense burst first: ~3.4us of continuous matmul activity
                # flips the HAM to K=8/8 before the transpose stream begins
                pump_burst(16)
                for i in range(ST):
                    phase_a_tile(x_dec_t, xd_d, i, enc=False)
                # chunks lag one s-tile so their inputs (the previous
                # tile's PSUM->SBUF copies) are already drained
                post = {0: [], 1: [vA[0]], 2: [vA[1], qc0[0]],
                        3: [vA[2]], 4: [vA[3], qc0[1]],
                        5: [vA[4], kc0[0]], 6: [vA[5]], 7: [vA[6]]}
                for i in range(ST):
                    phase_a_tile(x_enc_t, xe_d, i, enc=True)
                    for c in post[i]:
                        c()
                vA[7]()
                kc0[1]()
                k_s = k_s_next[0]
                v2 = v2_next[0]

            concat_t = []
            vch_cache = {}
            pending_pv = [None]

            def emit_pv(args):
                v2_, vc_, pt_ev_, pt_od_, rsTs_, concat_, blk_ = args
                ps_o = pstile()
                for kt_i in range(ST):
                    nc.tensor.matmul(
                        ps_o[0:64, 0:QB],
                        v2_[:, kt_i, vc_:vc_ + 64],
                        pt_ev_[:, kt_i, :],
                        start=(kt_i == 0), stop=(kt_i == ST - 1),
                        tile_position=(0, 0))
                    nc.tensor.matmul(
                        ps_o[64:128, 0:QB],
                        v2_[:, kt_i, vc_ + 64:vc_ + 128],
                        pt_od_[:, kt_i, :],
                        start=(kt_i == 0), stop=(kt_i == ST - 1),
                        tile_position=(0, 64))
                # broadcast 1/rowsum across the d-partitions into the
                # spare half of ps_o via the head-selector (K=2):
                # out[d, q] = recip[head(d), qtb][q]
                for qtb in range(QTB):
                    nc.tensor.matmul(
                        ps_o[:, QB + qtb * P:QB + (qtb + 1) * P],
                        sel2[0:2, :],
                        rsTs_[qtb][0:2, :],
                        start=True, stop=True,
                        skip_group_check=True)
                # stage the broadcast to SBUF (DVE can read only one
                # PSUM operand), then normalize in the PV->concat copy
                mult_sb = mlp.tile([P, QB], f16, tag="ml", name="ml")
                nc.scalar.activation(mult_sb[:], ps_o[:, QB:2 * QB],
                                     AF.Copy)
                for h01 in range(2):
                    hs = slice(h01 * 64, (h01 + 1) * 64)
                    nc.vector.tensor_tensor(
                        concat_[hs, blk_ * QB:(blk_ + 1) * QB],
                        ps_o[hs, 0:QB], mult_sb[hs, :],
                        op=OP.mult)

            wo_pre = {}

            def load_wo(sg):
                # one DMA per sg: [nd, 256 h_out cols] fp16, split by pair
                wt = worp.tile([P, HT, 2 * P], f16, tag="wor", name="wor")
                nc.gpsimd.dma_start(
                    wt[:],
                    wot_d[:, sg * 2 * P:(sg + 1) * 2 * P]
                    .rearrange("(j r) c -> r j c", j=HT))
                wo_pre[sg] = wt

            for p in range(NP):
                _sc = nc.named_scope(f"pair{p}")
                _sc.__enter__()
                if p + 2 < NP:
                    load_wt(q_wt, wqt_d, p + 2, "qw")
                    load_wt(k_wt, wkt_d, p + 2, "kw")
                chunks = []
                if p + 1 < NP:
                    chunks += q_chunks(p + 1)
                    chunks += k_chunks(p + 1)
                G = (p // 4 + 1) * 4
                if G < NP:
                    if G not in vch_cache:
                        vch_cache[G] = v_chunks(G)
                    chunks += [vch_cache[G][2 * (p % 4)],
                               vch_cache[G][2 * (p % 4) + 1]]
                vc = (p % 4) * P

                concat = ccp.tile([P, S], f16, tag="cc", name="concat")
                concat_t.append(concat)

                for blk in range(NB):
                    pt_ev = ptp.tile([P, ST, QB], f16, tag="pt", name="ptev")
                    pt_od = ptp.tile([P, ST, QB], f16, tag="pt", name="ptod")
                    # per-blk rowsum columns: [q, (h01*2+qtb)]
                    rs_blk = rsp.tile([P, 4], f32, tag="rs", name="rs")
                    for qtb in range(QTB):
                        qt = blk * QTB + qtb
                        qs = slice(qt * P, (qt + 1) * P)
                        ps_s = [pstileS(), pstileS()]
                        negmaxes = []
                        # h0/h1 contract K=64 in disjoint row groups (base
                        # partitions 0/64) -> the 4 score MMs overlap on PE
                        for h01 in range(2):
                            hs = slice(h01 * 64, h01 * 64 + 64)
                            for kk in range(2):
                                ks = slice(kk * 512, (kk + 1) * 512)
                                nc.tensor.matmul(
                                    ps_s[h01][:, ks],
                                    q_t[p][hs, qs], k_s[hs, ks],
                                    start=True, stop=True)
                            negmax = stat()
                            nc.vector.tensor_reduce(
                                negmax[:], ps_s[h01][:], axis=AX,
                                op=OP.max, negate=True)
                            negmaxes.append(negmax)
                        if pending_pv[0] is not None:
                            emit_pv(pending_pv[0])
                            pending_pv[0] = None
                        elif chunks:
                            chunks.pop(0)()
                        for h01 in range(2):
                            pt_dst = pt_ev if h01 == 0 else pt_od
                            p_e = pep.tile([P, S], f16, tag="pe")
                            nc.scalar.activation(
                                p_e[:], ps_s[h01][:], AF.Exp,
                                bias=negmaxes[h01][:],
                                accum_out=rs_blk[:, qtb * 2 + h01:
                                                 qtb * 2 + h01 + 1])
                            nc.sync.dma_start_transpose(
                                pt_dst[:, :, qtb * P:(qtb + 1) * P], p_e[:])
                            # insurance pump chained on p_e: keeps PE
                            # activity alive through the softmax chain
                            nc.tensor.matmul(
                                ps_s[h01][0:64, 0:P], p_e[:, 0:64],
                                p_e[:, 0:P], start=True, stop=True,
                                skip_group_check=True)
                    # batched recip -> fp16 -> tiny transposes: q to free
                    rc_blk = rsp.tile([P, 4], f32, tag="rc", name="rc")
                    nc.vector.reciprocal(rc_blk[:], rs_blk[:])
                    rsTs = []
                    for qtb in range(QTB):
                        rs16 = rs16_slots[(blk % 2) * 2 + qtb]
                        nc.vector.tensor_copy(
                            rs16[:, 0:2], rc_blk[:, qtb * 2:qtb * 2 + 2])
                        rsT = rstp.tile([P, P], f16, tag="rst", name="rst")
                        nc.sync.dma_start_transpose(rsT[:], rs16[:])
                        rsTs.append(rsT)
                    pending_pv[0] = (v2, vc, pt_ev, pt_od, rsTs, concat,
                                     blk)
                # drain leftover prep chunks; the last block's PV carries
                # into the next pair's first qt slot
                for c in chunks:
                    c()
                if p + 1 < NP:
                    k_s = k_s_next[0]
                    if (p + 1) % 4 == 0:
                        v2 = v2_next[0]
                _sc.__exit__(None, None, None)
            emit_pv(pending_pv[0])

            # ---- phase D: outT = W_out concat + b ----
            # outT[hout, q] accumulated over pairs with W_out slices
            # stationary (fp16, loaded one DMA per sg) and concat moving;
            # bias lands in PSUM via a K=1 ones-matmul; PSUM -> SBUF on
            # DVE/ACT, output DMA on 2 queues
            _scD = nc.named_scope("phaseD")
            _scD.__enter__()
            load_wo(0)
            load_wo(1)
            pump_burst(12)
            for sg in range(4):
                wt = wo_pre.pop(sg)
                if sg + 2 < 4:
                    load_wo(sg + 2)
                ps_big = [pstileS(pump=False), pstileS(pump=False)]
                for sl in range(2):
                    pump_into(ps_big[sl][:], 128)
                for p in range(NP):
                    for sl in range(2):
                        for qh in range(2):
                            nc.tensor.matmul(
                                ps_big[sl][:, qh * 512:(qh + 1) * 512],
                                wt[:, p, sl * P:(sl + 1) * P],
                                concat_t[p][:, qh * 512:(qh + 1) * 512],
                                start=(p == 0), stop=False,
                                skip_group_check=True)
                for sl in range(2):
                    ht = sg * 2 + sl
                    for qh in range(2):
                        # += bias_col . ones_q : broadcasts b_out along q
                        nc.tensor.matmul(
                            ps_big[sl][:, qh * 512:(qh + 1) * 512],
                            bias16[0:1, ht * P:(ht + 1) * P],
                            ones16[0:1, :],
                            start=False, stop=True,
                            skip_group_check=True)
                    osb = osbp.tile([P, S], f32, tag="osb", name="osb")
                    if sl == 0:
                        nc.vector.tensor_copy(osb[:], ps_big[sl][:])
                    else:
                        nc.scalar.activation(osb[:], ps_big[sl][:], AF.Copy)
                    eng = nc.sync if sl % 2 == 0 else nc.scalar
                    eng.dma_start(out_d[ht * P:(ht + 1) * P, :], osb[:])
            _scD.__exit__(None, None, None)

    nc.compile()
    return nc


def prep_in_maps(decoder_input, encoder_output, W_query, W_key, W_value,
                 W_out, b_out):
    f = lambda a: np.ascontiguousarray(np.asarray(a, dtype=np.float32))
    di = f(decoder_input)
    eo = f(encoder_output)
    wq = np.ascontiguousarray((f(W_query).reshape(H, H) * np.float32(0.125)).T)
    wk = np.ascontiguousarray(f(W_key).reshape(H, H).T)
    wv = np.ascontiguousarray(f(W_value).reshape(H, H).T.astype(np.float16))
    wo = np.ascontiguousarray(f(W_out).T.astype(np.float16))
    bias = np.ascontiguousarray(
        f(b_out).reshape(1, H).astype(np.float16))
    sel = np.zeros((2, P), dtype=np.float16)
    sel[0, 0:64] = 1.0
    sel[1, 64:128] = 1.0
    return [
        {"xd": di[b], "xe": eo[b], "wqt": wq, "wkt": wk, "wvt": wv,
         "wot": wo, "bias": bias, "sel": sel}
        for b in range(B)
    ]


_BUILT = None


def kernel(decoder_input, encoder_output, W_query, W_key, W_value, W_out,
           b_out):
    global _BUILT
    from concourse import bass_utils
    if _BUILT is None:
        _BUILT = build()
    in_maps = prep_in_maps(decoder_input, encoder_output, W_query, W_key,
                           W_value, W_out, b_out)
    try:
        res = bass_utils.run_bass_kernel_spmd(_BUILT, in_maps,
                                              core_ids=list(range(B)))
    except Exception:
        # one retry: a previously wedged NeuronCore can fail the first
        # execution after load
        res = bass_utils.run_bass_kernel_spmd(_BUILT, in_maps,
                                              core_ids=list(range(B)))
    # device returns out^T [h_out, q]; transpose back per batch element
    return np.stack([np.ascontiguousarray(res.results[b]["out"].T)
                     for b in range(B)], axis=0)
